# revision 54
# baseline (speedup 1.0000x reference)
"""DGCNN (gnn_message_passing) Trainium2 Bass kernel.

Strategy: graph-level data parallelism. 1024 graphs of 128 nodes / 2048 edges
are sharded 128-per-core across 8 NeuronCores. Per graph, the dense 128x128
adjacency count matrix M is built on the TensorEngine by a onehot-matmul over
the 2048 edges (onehots generated by DVE is_equal against an iota tile); the
4 GCN layers then become small dense matmuls with the symmetric normalization
folded in algebraically:

    o = dis * (Mbar^T (dis * (h @ W))) + b,   Mbar = M + I,  dis = deg^-1/2

Top-k(20) pooling is done with the DVE max8/max_index/match_replace
instructions batched over all graphs, selection + conv1d head as small
matmuls, and the final MLP batched with graphs on the partition dim.
"""

import numpy as np

import concourse.bass as bass
import concourse.tile as tile
from concourse import bacc, mybir
from concourse.bass_utils import run_bass_kernel_spmd

F32 = mybir.dt.float32
BF16 = mybir.dt.bfloat16
I32 = mybir.dt.int32
U32 = mybir.dt.uint32
AF = mybir.ActivationFunctionType
OP = mybir.AluOpType
AX = mybir.AxisListType

P = 128          # nodes per graph
D = 16           # avg degree -> edges per graph = P*D
EPG = P * D      # 2048 edges per graph
NF = 128         # node features
K = 20           # top-k
NCORES = 8
G_TOTAL = 1024


def _constants(inputs):
    """Host-side packing of weights/constants (layout only, no graph math)."""
    c = {}
    W1 = np.asarray(inputs["W1"], np.float32)          # [129, 32]
    c["W1a"] = np.ascontiguousarray(W1[0:128])          # [128, 32]
    c["W1b"] = np.ascontiguousarray(W1[128:129])        # [1, 32]
    c["W2"] = np.asarray(inputs["W2"], np.float32)
    c["W3"] = np.asarray(inputs["W3"], np.float32)
    c["W4"] = np.asarray(inputs["W4"], np.float32)      # [32, 1]
    c["b1c"] = np.asarray(inputs["b1"], np.float32).reshape(32, 1)
    c["b2c"] = np.asarray(inputs["b2"], np.float32).reshape(32, 1)
    c["b3c"] = np.asarray(inputs["b3"], np.float32).reshape(32, 1)
    b4 = np.asarray(inputs["b4"], np.float32).reshape(1, 1)
    c["b4c1"] = b4                                      # [1, 1]
    c["b4rep"] = np.repeat(b4, 128, axis=0)             # [128, 1]
    c1wT = np.asarray(inputs["c1_w"], np.float32).T  # [97, 16]
    c1p = np.zeros((32, 4, 16), np.float32)
    c1p[:, 0, :] = c1wT[0:32]
    c1p[:, 1, :] = c1wT[32:64]
    c1p[:, 2, :] = c1wT[64:96]
    c1p[0, 3, :] = c1wT[96]
    c["c1wT"] = c1p
    c["c1b_row"] = np.asarray(inputs["c1_b"], np.float32).reshape(1, 16)
    # c2_w [32 out, 16 in, 5 k] -> [ (k,i), out ] = [80, 32]
    c["c2wT"] = np.ascontiguousarray(
        np.asarray(inputs["c2_w"], np.float32).transpose(1, 2, 0)
    )  # [i=16, k=5, o=32]
    c["c2b_col"] = np.asarray(inputs["c2_b"], np.float32).reshape(32, 1)
    # out_w [(o*6+t), u] -> reorder rows to [(t*32+o), u]
    ow = np.asarray(inputs["out_w"], np.float32).reshape(32, 6, 1024)
    c["outw_r"] = np.ascontiguousarray(ow.transpose(1, 0, 2))  # [t=6, o=32, u] -> stored [32, 6, 1024]
    c["outw_r"] = np.ascontiguousarray(c["outw_r"].transpose(1, 0, 2))
    c["outb_row"] = np.asarray(inputs["out_b"], np.float32).reshape(1, 1024)
    c["h1w"] = np.ascontiguousarray(
        np.asarray(inputs["h1_w"], np.float32).reshape(8, 128, 128).transpose(1, 0, 2))
    c["h1b_row"] = np.asarray(inputs["h1_b"], np.float32).reshape(1, 128)
    c["h2w"] = np.asarray(inputs["h2_w"], np.float32)   # [128, 16]
    c["h2b_row"] = np.asarray(inputs["h2_b"], np.float32).reshape(1, 16)
    c["ident"] = np.eye(128, dtype=np.float32)
    c["W1b_r4"] = np.tile(c["W1b"], (4, 1))            # [4, 32]
    c["W2_r4"] = np.tile(c["W2"], (4, 1))              # [128, 32]
    c["W3_r4"] = np.tile(c["W3"], (4, 1))
    c["W4_r4"] = np.tile(c["W4"], (4, 1))              # [128, 1]
    c["c1wT_r4"] = np.tile(c["c1wT"], (4, 1, 1))       # [128, 4, 16]
    c["c1w96_r4"] = np.tile(c1wT[96].reshape(1, 16), (4, 1))  # [4, 16]
    c["b1c4"] = np.tile(c["b1c"], (4, 1))
    c["b2c4"] = np.tile(c["b2c"], (4, 1))
    c["b3c4"] = np.tile(c["b3c"], (4, 1))
    c["iota_i"] = np.broadcast_to(np.arange(128, dtype=np.int32), (128, 128)).copy()
    c["iota_f"] = np.broadcast_to(np.arange(128, dtype=np.float32), (128, 128)).copy()
    import ml_dtypes
    c["iota_b"] = c["iota_f"].astype(ml_dtypes.bfloat16)
    c["ones_row"] = np.ones((1, 128), np.float32)
    c["ones_col"] = np.ones((128, 1), np.float32)
    c["ident_b"] = np.eye(128, dtype=ml_dtypes.bfloat16)
    c["ident130_b"] = np.concatenate(
        [np.eye(128), np.zeros((128, 2))], axis=1).astype(ml_dtypes.bfloat16)
    c["ones_col_b"] = np.ones((128, 1), ml_dtypes.bfloat16)
    # row-selector weights: W1b_sel[g] = e_g (x) W1b, c1w96_sel[g] = e_g (x) c1_w[:,96]
    gpb = 8
    w1b = c["W1b"].reshape(32)
    sel = np.zeros((gpb, gpb, 32), np.float32)
    for g in range(gpb):
        sel[g, g, :] = w1b
    c["W1b_sel"] = sel
    c96 = np.asarray(inputs["c1_w"], np.float32).T[96].reshape(16)
    sel2 = np.zeros((gpb, gpb, 16), np.float32)
    for g in range(gpb):
        sel2[g, g, :] = c96
    c["c1w96_sel"] = sel2
    # stacked c1 weights: rows 0:96 = features of layers 1-3, row 96 = bias
    c["c1cat"] = np.concatenate(
        [c1wT[0:96], np.asarray(inputs["c1_b"], np.float32).reshape(1, 16)],
        axis=0)                                         # [97, 16]
    # W3/W4 staged at partition offsets 32/64 (matmul base-partition match)
    wpad = np.zeros((96, 32), np.float32)
    wpad[32:64, 0:32] = c["W3"]
    wpad[64:96, 0:1] = c["W4"]
    c["Wpad"] = wpad
    return c


CONST_SPECS = [
    ("W1a", [128, 32], F32), ("W1b", [1, 32], F32),
    ("W2", [32, 32], F32), ("W3", [32, 32], F32), ("W4", [32, 1], F32),
    ("b1c", [32, 1], F32), ("b2c", [32, 1], F32), ("b3c", [32, 1], F32),
    ("b4c1", [1, 1], F32), ("b4rep", [128, 1], F32),
    ("c1wT", [32, 4, 16], F32), ("c1b_row", [1, 16], F32),
    ("c2wT", [16, 5, 32], F32), ("c2b_col", [32, 1], F32),
    ("outw_r", [32, 6, 1024], F32), ("outb_row", [1, 1024], F32),
    ("h1w", [128, 8, 128], F32), ("h1b_row", [1, 128], F32),
    ("h2w", [128, 16], F32), ("h2b_row", [1, 16], F32),
    ("ident", [128, 128], F32),
    ("W1b_r4", [4, 32], F32), ("W2_r4", [128, 32], F32), ("W3_r4", [128, 32], F32),
    ("W4_r4", [128, 1], F32), ("c1wT_r4", [128, 4, 16], F32), ("c1w96_r4", [4, 16], F32),
    ("b1c4", [128, 1], F32), ("b2c4", [128, 1], F32), ("b3c4", [128, 1], F32),
    ("iota_i", [128, 128], I32), ("iota_f", [128, 128], F32),
    ("iota_b", [128, 128], BF16),
    ("ones_row", [1, 128], F32), ("ones_col", [128, 1], F32),
    ("ident_b", [128, 128], BF16), ("ident130_b", [128, 130], BF16),
    ("ones_col_b", [128, 1], BF16),
    ("W1b_sel", [8, 8, 32], F32), ("c1w96_sel", [8, 8, 16], F32),
    ("c1cat", [97, 16], F32), ("Wpad", [96, 32], F32),
]


def build(gpc=128, gpb=8, gpb2=16, unroll_static=True, n_pool_cmp=5,
          do_compile=True):
    """Build the per-core SPMD program. gpc = graphs per core, gpb = graphs
    per loop-1 body, gpb2 = graphs per loop-2 body, n_pool_cmp = number of
    the 32 per-graph onehot compares offloaded to the Pool engine."""
    assert gpc % gpb == 0 and gpc % gpb2 == 0
    groups = gpc // gpb
    nc = bacc.Bacc("TRN2", target_bir_lowering=False, debug=False)

    # xs packed host-side as [node, graph, feat] so one DMA pulls 4 graphs
    # with 2KB-contiguous partition lines
    xs_d = nc.dram_tensor("xs", [P, gpc * NF], F32, kind="ExternalInput")
    rowt_d = nc.dram_tensor("rowt", [groups * 128, gpb * 16], I32, kind="ExternalInput")
    colt_d = nc.dram_tensor("colt", [groups * 128, gpb * 16], I32, kind="ExternalInput")
    attrt_d = nc.dram_tensor("attrt", [groups * 128, gpb * 16], F32, kind="ExternalInput")
    cds = {n: nc.dram_tensor(n, s, dt, kind="ExternalInput") for n, s, dt in CONST_SPECS}
    out_d = nc.dram_tensor("out", [gpc, 16], F32, kind="ExternalOutput")

    with tile.TileContext(nc) as tc:
        with tc.tile_pool(name="const", bufs=1) as cpool, \
             tc.tile_pool(name="persist", bufs=1) as ppool:
            C = {}
            for n, s, dt in CONST_SPECS:
                C[n] = cpool.tile(s, dt, tag=f"c_{n}", name=f"c_{n}")
                nc.sync.dma_start(C[n][:], cds[n][:])

            c1all = ppool.tile([128, gpc, 16], F32, tag="c1all")
            C2all = ppool.tile([32, gpc, 6], F32, tag="C2all")

            # ---------------- loop 1: M build + GCN layers + c1 ----------------
            with tc.tile_pool(name="work", bufs=2) as wp, \
                 tc.tile_pool(name="psM", bufs=2, space="PSUM") as psM, \
                 tc.tile_pool(name="psT", bufs=2, space="PSUM") as psT, \
                 tc.tile_pool(name="psP", bufs=2, space="PSUM") as psP, \
                 tc.tile_pool(name="psQ", bufs=2, space="PSUM") as psQ:

                def body1(i):
                    # per-group edge loads ([128, gpb*16] tiles, contiguous in DRAM)
                    rowt = wp.tile([128, gpb * 16], I32, tag="rowt")
                    colt = wp.tile([128, gpb * 16], I32, tag="colt")
                    attrt = wp.tile([128, gpb * 16], F32, tag="attrt")
                    nc.sync.dma_start(rowt[:], rowt_d[bass.ds(i * 128, 128), :])
                    nc.sync.dma_start(colt[:], colt_d[bass.ds(i * 128, 128), :])
                    nc.sync.dma_start(attrt[:], attrt_d[bass.ds(i * 128, 128), :])
                    rowl = wp.tile([128, gpb * 16], I32, tag="rowl")
                    coll = wp.tile([128, gpb * 16], I32, tag="coll")
                    nc.vector.tensor_scalar(rowl[:], rowt[:], 127, None, op0=OP.bitwise_and)
                    nc.vector.tensor_scalar(coll[:], colt[:], 127, None, op0=OP.bitwise_and)
                    rowlf = wp.tile([128, gpb * 16], F32, tag="rowlf")
                    collf = wp.tile([128, gpb * 16], F32, tag="collf")
                    nc.vector.tensor_copy(rowlf[:], rowl[:])
                    nc.vector.tensor_copy(collf[:], coll[:])

                    G = range(gpb)
                    xg, soh, ta, mbar, xe, rec, dis, xt, xet = ({} for _ in range(9))
                    m_ps, xt_ps, xet_ps, phat = {}, {}, {}, {}
                    # stage: x loads (4 graphs per DMA; xs_d packed [n, g, f])
                    xg4 = {}
                    for q in range(gpb // 4):
                        xg4[q] = wp.tile([128, 4, 128], F32, tag="xg", name=f"xg{q}", bufs=2 * (gpb // 4))
                        nc.sync.dma_start(
                            xg4[q][:],
                            xs_d[:, bass.ds((i * gpb + 4 * q) * NF, 4 * NF)])
                    for gl in G:
                        xg[gl] = xg4[gl // 4][:, gl % 4, :]
                    # x transposes issued early: PE fills the compare-wait gap
                    for gl in G:
                        xt_ps[gl] = psT.tile([128, 128], F32, tag="T", name=f"xtps{gl}")
                        nc.tensor.transpose(xt_ps[gl][:], xg[gl], C["ident"][:])
                        xt[gl] = wp.tile([128, 128], F32, tag="xt", name=f"xt{gl}", bufs=gpb + 2)
                        nc.scalar.activation(xt[gl][:], xt_ps[gl][:], AF.Copy)
                    # stage: onehot compares. Pool handles the LAST matmul
                    # slots (emitted first for a head start) so the in-order
                    # PSUM accumulation is not head-blocked by slow Pool ops.
                    pool_js = set(range(16 - n_pool_cmp, 16))
                    for gl in G:
                        soh[gl] = wp.tile([128, 16, 128], BF16, tag="soh", name=f"soh{gl}", bufs=gpb + 2)
                        ta[gl] = wp.tile([128, 16, 130], BF16, tag="ta", name=f"ta{gl}", bufs=gpb + 2)
                        for j in sorted(range(16), key=lambda j: (j not in pool_js, j)):
                            e = 16 * gl + j
                            eng_s = nc.gpsimd if j in pool_js else nc.vector
                            eng_s.tensor_scalar(
                                soh[gl][:, j, :], C["iota_b"][:], rowlf[:, e:e + 1], None,
                                op0=OP.is_equal)
                            nc.vector.tensor_scalar(
                                ta[gl][:, j, 0:128], C["iota_b"][:], collf[:, e:e + 1], None,
                                op0=OP.is_equal)
                        nc.vector.tensor_copy(ta[gl][:, :, 128], attrt[:, 16 * gl:16 * gl + 16])
                    # stage: M accumulation (identity folded in as an extra
                    # matmul -> PSUM holds M+I directly) + deg + dis
                    for gl in G:
                        m_ps[gl] = psM.tile([128, 130], F32, tag="M", name=f"mps{gl}")
                        nc.tensor.matmul(m_ps[gl][:, 0:129], C["ident_b"][:],
                                         C["ident130_b"][:, 0:129],
                                         start=True, stop=False)
                        for j in sorted(range(16), key=lambda j: (j in pool_js, j)):
                            nc.tensor.matmul(m_ps[gl][:, 0:129], soh[gl][:, j, :],
                                             ta[gl][:, j, 0:129],
                                             start=False, stop=(j == 15))
                    xe8 = wp.tile([128, gpb], F32, tag="xe8", name="xe8", bufs=3)
                    o4c8 = wp.tile([128, gpb], F32, tag="o4c8", name="o4c8", bufs=3)
                    deg8_ps = psQ.tile([128, 32], F32, tag="Qp", name="deg8ps")
                    for gl in G:
                        mbar[gl] = wp.tile([128, 128], F32, tag="mbar", name=f"mbar{gl}", bufs=gpb + 2)
                        nc.scalar.activation(mbar[gl][:], m_ps[gl][:, 0:128], AF.Copy)
                        nc.scalar.activation(xe8[:, gl:gl + 1], m_ps[gl][:, 128:129], AF.Copy)
                        nc.tensor.matmul(deg8_ps[:, gl:gl + 1], mbar[gl][:], C["ones_col"][:],
                                         start=True, stop=True)
                    rec8 = wp.tile([128, gpb], F32, tag="rec8", name="rec8", bufs=3)
                    nc.vector.reciprocal(rec8[:], deg8_ps[:, 0:gpb])
                    dis8 = wp.tile([128, gpb], F32, tag="dis8", name="dis8", bufs=3)
                    nc.scalar.activation(dis8[:], rec8[:], AF.Sqrt)
                    for gl in G:
                        dis[gl] = dis8[:, gl:gl + 1]
                        nc.gpsimd.tensor_scalar(mbar[gl][:], mbar[gl][:], dis[gl], None,
                                                op0=OP.mult)
                    BW = 4 if gpb % 4 == 0 else (2 if gpb % 2 == 0 else 1)
                    NB = gpb // BW
                    # batched transpose of all xe columns -> one [gpb,128] copy
                    xet8_ps = psT.tile([128, 128], F32, tag="T", name="xet8ps")
                    nc.tensor.transpose(xet8_ps[0:gpb, :], xe8[:], C["ident"][:])
                    xet8 = wp.tile([gpb, 128], F32, tag="xet8", name="xet8", bufs=3)
                    nc.scalar.activation(xet8[:], xet8_ps[0:gpb, :], AF.Copy)
                    # stage: P1 (raw, no dis scaling -- dis lives in mbar rows)
                    phat4 = {}
                    for b in range(NB):
                        p_ps = psP.tile([128, 32 * BW], F32, tag="Pp", name=f"pps{b}")
                        for m in range(BW):
                            gl = b * BW + m
                            nc.tensor.matmul(p_ps[:, 32 * m:32 * m + 32], xt[gl][:],
                                             C["W1a"][:], start=True, stop=False)
                            nc.tensor.matmul(p_ps[:, 32 * m:32 * m + 32], xet8[:],
                                             C["W1b_sel"][:, gl, :], start=False, stop=True)
                        phat4[b] = wp.tile([128, 32 * BW], F32, tag="phat", name=f"phat{b}", bufs=gpb + 2)
                        nc.scalar.activation(phat4[b][:], p_ps[:], AF.Copy)
                    # layers, stage-interleaved across graphs
                    pw = [C["W2"][:], C["Wpad"][32:64, 0:32], C["Wpad"][64:96, 0:1]]
                    bcols = [C["b1c"], C["b2c"], C["b3c"]]
                    oitall = {}
                    for b in range(NB):
                        oitall[b] = wp.tile([128, 128 * BW], F32, tag="oitall",
                                            name=f"oitall{b}", bufs=NB + 1)
                        nc.gpsimd.memset(oitall[b][96:97, :], 1.0)
                    q_ps, q1 = {}, {}
                    for li in range(4):
                        w = 32 if li < 3 else 1
                        for gl in G:
                            b, m = gl // BW, gl % BW
                            q_ps[gl] = psQ.tile([128, 32], F32, tag="Qp", name=f"qps{li}_{gl}")
                            nc.tensor.matmul(q_ps[gl][:, 0:w], mbar[gl][:],
                                             phat4[b][:, w * m:w * m + w],
                                             start=True, stop=True)
                            if li < 3:
                                q1[gl] = wp.tile([128, 32], F32, tag="q1", name=f"q1_{li}_{gl}", bufs=gpb + 2)
                                nc.scalar.activation(q1[gl][:, 0:w], q_ps[gl][:, 0:w], AF.Copy,
                                                     scale=dis[gl])
                            else:
                                # o4 (sans +b4: constant shift, rank-invariant)
                                # into the group score-column tile
                                nc.scalar.activation(o4c8[:, gl:gl + 1],
                                                     q_ps[gl][:, 0:1], AF.Copy,
                                                     scale=dis[gl])
                        if li < 3:
                            for b in range(NB):
                                q1t_ps = psT.tile([32, 128 * BW], F32, tag="T", name=f"q1tps{li}_{b}")
                                for m in range(BW):
                                    nc.tensor.transpose(q1t_ps[0:32, 128 * m:128 * m + 128],
                                                        q1[b * BW + m][:, 0:32], C["ident"][:])
                                nc.scalar.activation(oitall[b][32 * li:32 * li + 32, :],
                                                     q1t_ps[:], AF.Identity,
                                                     bias=bcols[li][:])
                            for b in range(NB):
                                p_ps2 = psP.tile([128, 32 * BW], F32, tag="Pp", name=f"pps2{li}_{b}")
                                wnext = 32 if li < 2 else 1
                                for m in range(BW):
                                    nc.tensor.matmul(p_ps2[:, wnext * m:wnext * m + wnext],
                                                     oitall[b][32 * li:32 * li + 32, 128 * m:128 * m + 128],
                                                     pw[li], start=True, stop=True)
                                phat4[b] = wp.tile([128, 32 * BW], F32, tag="phat", name=f"phatL{li}_{b}", bufs=gpb + 2)
                                nc.scalar.activation(phat4[b][:, 0:wnext * BW], p_ps2[:, 0:wnext * BW], AF.Copy)
                        else:
                            # batched transpose of the gpb o4 columns, one
                            # bias-adding copy for all graphs in the group
                            o4t8_ps = psT.tile([128, 128], F32, tag="T", name="o4t8ps")
                            nc.tensor.transpose(o4t8_ps[0:gpb, :], o4c8[:],
                                                C["ident"][:])
                            o4t8 = wp.tile([gpb, 128], F32, tag="o4t8", name="o4t8", bufs=3)
                            nc.scalar.activation(o4t8[:], o4t8_ps[0:gpb, :],
                                                 AF.Identity, bias=C["b4rep"][0:gpb, :])
                    # stage: c1 over all nodes + relu (one stacked matmul:
                    # 96 layer-feature rows + ones/bias row, plus o4 select)
                    for gl in G:
                        b, m = gl // BW, gl % BW
                        c1_ps = psQ.tile([128, 32], F32, tag="Qp", name=f"c1ps{gl}")
                        nc.tensor.matmul(c1_ps[:, 0:16],
                                         oitall[b][0:97, 128 * m:128 * m + 128],
                                         C["c1cat"][:], start=True, stop=False)
                        nc.tensor.matmul(c1_ps[:, 0:16], o4t8[:],
                                         C["c1w96_sel"][:, gl, :], start=False, stop=True)
                        nc.scalar.activation(c1all[:, bass.ds(i * gpb + gl, 1), :],
                                             c1_ps[:, 0:16], AF.Relu)

                    # ---- fused per-group top-k + select + maxpool + conv ----
                    scur = o4t8
                    idxf8 = wp.tile([gpb, 24], F32, tag="idxf8", name="idxf8", bufs=3)
                    for r in range(3):
                        mx = wp.tile([gpb, 8], F32, tag=f"mx{r}", name=f"mx{r}", bufs=3)
                        mi = wp.tile([gpb, 8], U32, tag=f"mi{r}", name=f"mi{r}", bufs=3)
                        nc.vector.max(mx[:], scur[:])
                        nc.vector.max_index(mi[:], mx[:], scur[:])
                        nc.vector.tensor_copy(idxf8[:, 8 * r:8 * r + 8], mi[:])
                        if r < 2:
                            snew = wp.tile([gpb, 128], F32, tag=f"sc{r}", name=f"sc{r}", bufs=3)
                            nc.vector.match_replace(snew[:], mx[:], scur[:], -1e30)
                            scur = snew
                    idxt8_ps = psT.tile([128, 128], F32, tag="T", name="idxt8ps")
                    nc.tensor.transpose(idxt8_ps[0:24, 0:gpb], idxf8[:],
                                        C["ident"][0:gpb, 0:gpb])
                    idxt8 = wp.tile([K, gpb], F32, tag="idxt8", name="idxt8", bufs=3)
                    nc.scalar.activation(idxt8[:], idxt8_ps[0:K, 0:gpb], AF.Copy)
                    selT, sel = {}, {}
                    for gl in G:
                        selT[gl] = wp.tile([K, 128], F32, tag="selT", name=f"selT{gl}", bufs=gpb + 2)
                        nc.vector.tensor_scalar(selT[gl][:], C["iota_f"][0:K, :],
                                                idxt8[:, gl:gl + 1], None,
                                                op0=OP.is_equal)
                    for gl in G:
                        sel_ps = psT.tile([128, 128], F32, tag="T", name=f"selps{gl}")
                        nc.tensor.transpose(sel_ps[:, 0:K], selT[gl][:], C["ident"][0:K, 0:K])
                        sel[gl] = wp.tile([128, K], F32, tag="sel", name=f"sel{gl}", bufs=gpb + 2)
                        nc.scalar.activation(sel[gl][:], sel_ps[:, 0:K], AF.Copy)
                    # pooled features, already transposed: [16, K] per graph
                    c1st_all = wp.tile([16, gpb, K], F32, tag="c1st", name="c1st_all", bufs=3)
                    for gl in G:
                        c1st_ps = psQ.tile([128, 32], F32, tag="Qp", name=f"c1stps{gl}")
                        nc.tensor.matmul(c1st_ps[0:16, 0:K],
                                         c1all[:, bass.ds(i * gpb + gl, 1), :],
                                         sel[gl][:], start=True, stop=True)
                        nc.scalar.activation(c1st_all[:, gl, :], c1st_ps[0:16, 0:K], AF.Copy)
                    # maxpool over pairs for all graphs in the group at once
                    mp_all = wp.tile([16, gpb, K // 2], F32, tag="mp", name="mp_all", bufs=3)
                    nc.vector.tensor_tensor(mp_all[:], c1st_all[:, :, 0:K:2],
                                            c1st_all[:, :, 1:K:2], op=OP.max)
                    # conv1d over the group: 5 batched matmuls
                    c2_ps = psP.tile([32, gpb, 6], F32, tag="Pp", name="c2ps")
                    for k5 in range(5):
                        nc.tensor.matmul(c2_ps[:], C["c2wT"][:, k5, :],
                                         mp_all[:, :, k5:k5 + 6],
                                         start=(k5 == 0), stop=(k5 == 4))
                    nc.scalar.activation(C2all[:, bass.ds(i * gpb, gpb), :], c2_ps[:],
                                         AF.Relu, bias=C["c2b_col"][:])

                if unroll_static:
                    for i in range(groups):
                        body1(i)
                else:
                    with tc.For_i(0, groups, 1) as i:
                        body1(i)

            # ---------------- batched MLP head ----------------
            with tc.tile_pool(name="mlp", bufs=1) as ml, \
                 tc.tile_pool(name="psD", bufs=2, space="PSUM") as psD, \
                 tc.tile_pool(name="psE", bufs=2, space="PSUM") as psE:
                d1_ps = [psD.tile([128, 512], F32, tag="d1", name=f"d1ps{h}") for h in range(2)]
                for h in range(2):
                    for t in range(6):
                        nc.tensor.matmul(d1_ps[h][0:gpc, :], C2all[:, :, t],
                                         C["outw_r"][:, t, 512 * h:512 * h + 512],
                                         start=(t == 0), stop=False)
                    nc.tensor.matmul(d1_ps[h][0:gpc, :], C["ones_row"][:, 0:gpc],
                                     C["outb_row"][:, 512 * h:512 * h + 512],
                                     start=False, stop=True)
                d1r = ml.tile([128, 1024], F32, tag="d1r")
                for h in range(2):
                    nc.scalar.activation(d1r[0:gpc, 512 * h:512 * h + 512],
                                         d1_ps[h][0:gpc, :], AF.Relu)
                d2_ps = psD.tile([128, 512], F32, tag="d1")
                for u in range(8):
                    t_ps = psE.tile([128, 128], F32, tag="tr")
                    nc.tensor.transpose(t_ps[0:128, 0:gpc],
                                        d1r[0:gpc, 128 * u:128 * u + 128],
                                        C["ident"][0:gpc, 0:gpc])
                    d1tu = ml.tile([128, 128], F32, tag=f"d1t{u % 2}")
                    nc.scalar.activation(d1tu[:, 0:gpc], t_ps[:, 0:gpc], AF.Copy)
                    nc.tensor.matmul(d2_ps[:, 0:gpc], C["h1w"][:, u, :],
                                     d1tu[:, 0:gpc], start=(u == 0), stop=False)
                nc.tensor.matmul(d2_ps[:, 0:gpc], C["h1b_row"][:], C["ones_row"][:, 0:gpc],
                                 start=False, stop=True)
                d2t = ml.tile([128, 128], F32, tag="d2t")
                nc.scalar.activation(d2t[:, 0:gpc], d2_ps[:, 0:gpc], AF.Relu)
                lg_ps = psE.tile([128, 128], F32, tag="tr")
                nc.tensor.matmul(lg_ps[0:gpc, 0:16], d2t[:, 0:gpc], C["h2w"][:],
                                 start=True, stop=False)
                nc.tensor.matmul(lg_ps[0:gpc, 0:16], C["ones_row"][:, 0:gpc],
                                 C["h2b_row"][:], start=False, stop=True)
                # log_softmax over the 16 logits
                negmax = ml.tile([128, 1], F32, tag="negmax")
                nc.vector.tensor_reduce(negmax[0:gpc, :], lg_ps[0:gpc, 0:16], axis=AX.X,
                                        op=OP.max, negate=True)
                ex = ml.tile([128, 16], F32, tag="ex")
                nc.scalar.activation(ex[0:gpc, :], lg_ps[0:gpc, 0:16], AF.Exp,
                                     bias=negmax[0:gpc, :])
                ssum = ml.tile([128, 1], F32, tag="ssum")
                nc.vector.tensor_reduce(ssum[0:gpc, :], ex[0:gpc, :], axis=AX.X, op=OP.add)
                lsum = ml.tile([128, 1], F32, tag="lsum")
                nc.scalar.activation(lsum[0:gpc, :], ssum[0:gpc, :], AF.Ln)
                mpls = ml.tile([128, 1], F32, tag="mpls")
                nc.vector.tensor_sub(mpls[0:gpc, :], lsum[0:gpc, :], negmax[0:gpc, :])
                outsb = ml.tile([128, 16], F32, tag="outsb")
                nc.vector.tensor_scalar(outsb[0:gpc, :], lg_ps[0:gpc, 0:16],
                                        mpls[0:gpc, :], None, op0=OP.subtract)
                nc.sync.dma_start(out_d[:], outsb[0:gpc, :])

    if do_compile:
        nc.compile()
    return nc


def shard_inputs(inputs, gpc=128, gpb=8):
    """Slice full inputs into 8 per-core input maps (layout packing only)."""
    x = np.asarray(inputs["x"], np.float32)
    ea = np.asarray(inputs["edge_attr"], np.float32).reshape(-1)
    ei = np.asarray(inputs["edge_index"], np.int32)
    consts = _constants(inputs)
    groups = gpc // gpb
    epc = gpc * EPG
    maps = []
    for c in range(NCORES):
        m = dict(consts)
        xc = x[c * gpc * P:(c + 1) * gpc * P]            # [gpc*128, 128]
        m["xs"] = np.ascontiguousarray(
            xc.reshape(gpc, P, NF).transpose(1, 0, 2).reshape(P, gpc * NF))
        for name, arr in (("rowt", ei[0]), ("colt", ei[1]), ("attrt", ea)):
            sl = arr[c * epc:(c + 1) * epc]
            # [groups, gpb, 16, 128] -> [groups, 128, gpb*16]
            t = sl.reshape(groups, gpb * 16, 128).transpose(0, 2, 1)
            m[name] = np.ascontiguousarray(t.reshape(groups * 128, gpb * 16))
        maps.append(m)
    return maps


_CACHE = {}


def kernel(**inputs) -> np.ndarray:
    gpc = 128
    key = ("full", gpc)
    if key not in _CACHE:
        _CACHE[key] = build(gpc=gpc)
    nc = _CACHE[key]
    maps = shard_inputs(inputs, gpc=gpc)
    res = run_bass_kernel_spmd(nc, maps, core_ids=list(range(NCORES)))
    return np.concatenate([res.results[i]["out"] for i in range(NCORES)], axis=0)



# revision 55
# speedup vs baseline: 1.0263x; 1.0263x over previous
"""DGCNN (gnn_message_passing) Trainium2 Bass kernel.

Strategy: graph-level data parallelism. 1024 graphs of 128 nodes / 2048 edges
are sharded 128-per-core across 8 NeuronCores. Per graph, the dense 128x128
adjacency count matrix M is built on the TensorEngine by a onehot-matmul over
the 2048 edges (onehots generated by DVE is_equal against an iota tile); the
4 GCN layers then become small dense matmuls with the symmetric normalization
folded in algebraically:

    o = dis * (Mbar^T (dis * (h @ W))) + b,   Mbar = M + I,  dis = deg^-1/2

Top-k(20) pooling is done with the DVE max8/max_index/match_replace
instructions batched over all graphs, selection + conv1d head as small
matmuls, and the final MLP batched with graphs on the partition dim.
"""

import numpy as np

import concourse.bass as bass
import concourse.tile as tile
from concourse import bacc, mybir
from concourse.bass_utils import run_bass_kernel_spmd

F32 = mybir.dt.float32
BF16 = mybir.dt.bfloat16
I32 = mybir.dt.int32
U32 = mybir.dt.uint32
AF = mybir.ActivationFunctionType
OP = mybir.AluOpType
AX = mybir.AxisListType

P = 128          # nodes per graph
D = 16           # avg degree -> edges per graph = P*D
EPG = P * D      # 2048 edges per graph
NF = 128         # node features
K = 20           # top-k
NCORES = 8
G_TOTAL = 1024


def _constants(inputs):
    """Host-side packing of weights/constants (layout only, no graph math)."""
    c = {}
    W1 = np.asarray(inputs["W1"], np.float32)          # [129, 32]
    c["W1a"] = np.ascontiguousarray(W1[0:128])          # [128, 32]
    c["W1b"] = np.ascontiguousarray(W1[128:129])        # [1, 32]
    c["W2"] = np.asarray(inputs["W2"], np.float32)
    c["W3"] = np.asarray(inputs["W3"], np.float32)
    c["W4"] = np.asarray(inputs["W4"], np.float32)      # [32, 1]
    c["b1c"] = np.asarray(inputs["b1"], np.float32).reshape(32, 1)
    c["b2c"] = np.asarray(inputs["b2"], np.float32).reshape(32, 1)
    c["b3c"] = np.asarray(inputs["b3"], np.float32).reshape(32, 1)
    b4 = np.asarray(inputs["b4"], np.float32).reshape(1, 1)
    c["b4c1"] = b4                                      # [1, 1]
    c["b4rep"] = np.repeat(b4, 128, axis=0)             # [128, 1]
    c1wT = np.asarray(inputs["c1_w"], np.float32).T  # [97, 16]
    c1p = np.zeros((32, 4, 16), np.float32)
    c1p[:, 0, :] = c1wT[0:32]
    c1p[:, 1, :] = c1wT[32:64]
    c1p[:, 2, :] = c1wT[64:96]
    c1p[0, 3, :] = c1wT[96]
    c["c1wT"] = c1p
    c["c1b_row"] = np.asarray(inputs["c1_b"], np.float32).reshape(1, 16)
    # c2_w [32 out, 16 in, 5 k] -> [ (k,i), out ] = [80, 32]
    c["c2wT"] = np.ascontiguousarray(
        np.asarray(inputs["c2_w"], np.float32).transpose(1, 2, 0)
    )  # [i=16, k=5, o=32]
    c["c2b_col"] = np.asarray(inputs["c2_b"], np.float32).reshape(32, 1)
    # out_w [(o*6+t), u] -> reorder rows to [(t*32+o), u]
    ow = np.asarray(inputs["out_w"], np.float32).reshape(32, 6, 1024)
    c["outw_r"] = np.ascontiguousarray(ow.transpose(1, 0, 2))  # [t=6, o=32, u] -> stored [32, 6, 1024]
    c["outw_r"] = np.ascontiguousarray(c["outw_r"].transpose(1, 0, 2))
    c["outb_row"] = np.asarray(inputs["out_b"], np.float32).reshape(1, 1024)
    c["h1w"] = np.ascontiguousarray(
        np.asarray(inputs["h1_w"], np.float32).reshape(8, 128, 128).transpose(1, 0, 2))
    c["h1b_row"] = np.asarray(inputs["h1_b"], np.float32).reshape(1, 128)
    c["h2w"] = np.asarray(inputs["h2_w"], np.float32)   # [128, 16]
    c["h2b_row"] = np.asarray(inputs["h2_b"], np.float32).reshape(1, 16)
    c["ident"] = np.eye(128, dtype=np.float32)
    c["W1b_r4"] = np.tile(c["W1b"], (4, 1))            # [4, 32]
    c["W2_r4"] = np.tile(c["W2"], (4, 1))              # [128, 32]
    c["W3_r4"] = np.tile(c["W3"], (4, 1))
    c["W4_r4"] = np.tile(c["W4"], (4, 1))              # [128, 1]
    c["c1wT_r4"] = np.tile(c["c1wT"], (4, 1, 1))       # [128, 4, 16]
    c["c1w96_r4"] = np.tile(c1wT[96].reshape(1, 16), (4, 1))  # [4, 16]
    c["b1c4"] = np.tile(c["b1c"], (4, 1))
    c["b2c4"] = np.tile(c["b2c"], (4, 1))
    c["b3c4"] = np.tile(c["b3c"], (4, 1))
    c["iota_i"] = np.broadcast_to(np.arange(128, dtype=np.int32), (128, 128)).copy()
    c["iota_f"] = np.broadcast_to(np.arange(128, dtype=np.float32), (128, 128)).copy()
    import ml_dtypes
    c["iota_b"] = c["iota_f"].astype(ml_dtypes.bfloat16)
    c["ones_row"] = np.ones((1, 128), np.float32)
    c["ones_col"] = np.ones((128, 1), np.float32)
    c["ident_b"] = np.eye(128, dtype=ml_dtypes.bfloat16)
    c["ident130_b"] = np.concatenate(
        [np.eye(128), np.zeros((128, 2))], axis=1).astype(ml_dtypes.bfloat16)
    c["ones_col_b"] = np.ones((128, 1), ml_dtypes.bfloat16)
    # row-selector weights: W1b_sel[g] = e_g (x) W1b, c1w96_sel[g] = e_g (x) c1_w[:,96]
    gpb = 8
    w1b = c["W1b"].reshape(32)
    sel = np.zeros((gpb, gpb, 32), np.float32)
    for g in range(gpb):
        sel[g, g, :] = w1b
    c["W1b_sel"] = sel
    c96 = np.asarray(inputs["c1_w"], np.float32).T[96].reshape(16)
    sel2 = np.zeros((gpb, gpb, 16), np.float32)
    for g in range(gpb):
        sel2[g, g, :] = c96
    c["c1w96_sel"] = sel2
    # stacked c1 weights: rows 0:96 = features of layers 1-3, row 96 = bias
    c["c1cat"] = np.concatenate(
        [c1wT[0:96], np.asarray(inputs["c1_b"], np.float32).reshape(1, 16)],
        axis=0)                                         # [97, 16]
    # W3/W4 staged at partition offsets 32/64 (matmul base-partition match)
    wpad = np.zeros((96, 32), np.float32)
    wpad[32:64, 0:32] = c["W3"]
    wpad[64:96, 0:1] = c["W4"]
    c["Wpad"] = wpad
    return c


CONST_SPECS = [
    ("W1a", [128, 32], F32), ("W1b", [1, 32], F32),
    ("W2", [32, 32], F32), ("W3", [32, 32], F32), ("W4", [32, 1], F32),
    ("b1c", [32, 1], F32), ("b2c", [32, 1], F32), ("b3c", [32, 1], F32),
    ("b4c1", [1, 1], F32), ("b4rep", [128, 1], F32),
    ("c1wT", [32, 4, 16], F32), ("c1b_row", [1, 16], F32),
    ("c2wT", [16, 5, 32], F32), ("c2b_col", [32, 1], F32),
    ("outw_r", [32, 6, 1024], F32), ("outb_row", [1, 1024], F32),
    ("h1w", [128, 8, 128], F32), ("h1b_row", [1, 128], F32),
    ("h2w", [128, 16], F32), ("h2b_row", [1, 16], F32),
    ("ident", [128, 128], F32),
    ("W1b_r4", [4, 32], F32), ("W2_r4", [128, 32], F32), ("W3_r4", [128, 32], F32),
    ("W4_r4", [128, 1], F32), ("c1wT_r4", [128, 4, 16], F32), ("c1w96_r4", [4, 16], F32),
    ("b1c4", [128, 1], F32), ("b2c4", [128, 1], F32), ("b3c4", [128, 1], F32),
    ("iota_i", [128, 128], I32), ("iota_f", [128, 128], F32),
    ("iota_b", [128, 128], BF16),
    ("ones_row", [1, 128], F32), ("ones_col", [128, 1], F32),
    ("ident_b", [128, 128], BF16), ("ident130_b", [128, 130], BF16),
    ("ones_col_b", [128, 1], BF16),
    ("W1b_sel", [8, 8, 32], F32), ("c1w96_sel", [8, 8, 16], F32),
    ("c1cat", [97, 16], F32), ("Wpad", [96, 32], F32),
]


def build(gpc=128, gpb=8, gpb2=16, unroll_static=True, n_pool_cmp=5,
          do_compile=True):
    """Build the per-core SPMD program. gpc = graphs per core, gpb = graphs
    per loop-1 body, gpb2 = graphs per loop-2 body, n_pool_cmp = number of
    the 32 per-graph onehot compares offloaded to the Pool engine."""
    assert gpc % gpb == 0 and gpc % gpb2 == 0
    groups = gpc // gpb
    nc = bacc.Bacc("TRN2", target_bir_lowering=False, debug=False)

    # xs packed host-side as [node, graph, feat] so one DMA pulls 4 graphs
    # with 2KB-contiguous partition lines
    xs_d = nc.dram_tensor("xs", [P, gpc * NF], F32, kind="ExternalInput")
    rowt_d = nc.dram_tensor("rowt", [groups * 128, gpb * 16], I32, kind="ExternalInput")
    colt_d = nc.dram_tensor("colt", [groups * 128, gpb * 16], I32, kind="ExternalInput")
    attrt_d = nc.dram_tensor("attrt", [groups * 128, gpb * 16], F32, kind="ExternalInput")
    cds = {n: nc.dram_tensor(n, s, dt, kind="ExternalInput") for n, s, dt in CONST_SPECS}
    out_d = nc.dram_tensor("out", [gpc, 16], F32, kind="ExternalOutput")

    with tile.TileContext(nc) as tc:
        with tc.tile_pool(name="const", bufs=1) as cpool, \
             tc.tile_pool(name="persist", bufs=1) as ppool:
            C = {}
            for n, s, dt in CONST_SPECS:
                C[n] = cpool.tile(s, dt, tag=f"c_{n}", name=f"c_{n}")
                nc.sync.dma_start(C[n][:], cds[n][:])

            c1all = ppool.tile([128, gpc, 16], F32, tag="c1all")
            C2all = ppool.tile([32, gpc, 6], F32, tag="C2all")

            # ---------------- loop 1: M build + GCN layers + c1 ----------------
            with tc.tile_pool(name="work", bufs=2) as wp, \
                 tc.tile_pool(name="psM", bufs=2, space="PSUM") as psM, \
                 tc.tile_pool(name="psT", bufs=2, space="PSUM") as psT, \
                 tc.tile_pool(name="psP", bufs=2, space="PSUM") as psP, \
                 tc.tile_pool(name="psQ", bufs=2, space="PSUM") as psQ:

                def body1(i):
                    # per-group edge loads ([128, gpb*16] tiles, contiguous in DRAM)
                    rowt = wp.tile([128, gpb * 16], I32, tag="rowt")
                    colt = wp.tile([128, gpb * 16], I32, tag="colt")
                    attrt = wp.tile([128, gpb * 16], F32, tag="attrt")
                    nc.sync.dma_start(rowt[:], rowt_d[bass.ds(i * 128, 128), :])
                    nc.sync.dma_start(colt[:], colt_d[bass.ds(i * 128, 128), :])
                    nc.sync.dma_start(attrt[:], attrt_d[bass.ds(i * 128, 128), :])
                    rowl = wp.tile([128, gpb * 16], I32, tag="rowl")
                    coll = wp.tile([128, gpb * 16], I32, tag="coll")
                    nc.vector.tensor_scalar(rowl[:], rowt[:], 127, None, op0=OP.bitwise_and)
                    nc.vector.tensor_scalar(coll[:], colt[:], 127, None, op0=OP.bitwise_and)
                    rowlf = wp.tile([128, gpb * 16], F32, tag="rowlf")
                    collf = wp.tile([128, gpb * 16], F32, tag="collf")
                    nc.vector.tensor_copy(rowlf[:], rowl[:])
                    nc.vector.tensor_copy(collf[:], coll[:])

                    G = range(gpb)
                    xg, soh, ta, mbar, xe, rec, dis, xt, xet = ({} for _ in range(9))
                    m_ps, xt_ps, xet_ps, phat = {}, {}, {}, {}
                    # stage: x loads (4 graphs per DMA; xs_d packed [n, g, f])
                    xg4 = {}
                    for q in range(gpb // 4):
                        xg4[q] = wp.tile([128, 4, 128], F32, tag="xg", name=f"xg{q}", bufs=2 * (gpb // 4))
                        nc.sync.dma_start(
                            xg4[q][:],
                            xs_d[:, bass.ds((i * gpb + 4 * q) * NF, 4 * NF)])
                    for gl in G:
                        xg[gl] = xg4[gl // 4][:, gl % 4, :]
                    # stage: onehot compares. Pool handles the LAST matmul
                    # slots (emitted first for a head start) so the in-order
                    # PSUM accumulation is not head-blocked by slow Pool ops.
                    pool_js = set(range(16 - n_pool_cmp, 16))
                    for gl in G:
                        soh[gl] = wp.tile([128, 16, 128], BF16, tag="soh", name=f"soh{gl}", bufs=gpb + 2)
                        ta[gl] = wp.tile([128, 16, 130], BF16, tag="ta", name=f"ta{gl}", bufs=gpb + 2)
                        for j in sorted(range(16), key=lambda j: (j not in pool_js, j)):
                            e = 16 * gl + j
                            eng_s = nc.gpsimd if j in pool_js else nc.vector
                            eng_s.tensor_scalar(
                                soh[gl][:, j, :], C["iota_b"][:], rowlf[:, e:e + 1], None,
                                op0=OP.is_equal)
                            nc.vector.tensor_scalar(
                                ta[gl][:, j, 0:128], C["iota_b"][:], collf[:, e:e + 1], None,
                                op0=OP.is_equal)
                        nc.vector.tensor_copy(ta[gl][:, :, 128], attrt[:, 16 * gl:16 * gl + 16])
                    # stage: M accumulation (identity folded in as an extra
                    # matmul -> PSUM holds M+I directly) + deg + dis
                    for gl in G:
                        m_ps[gl] = psM.tile([128, 130], F32, tag="M", name=f"mps{gl}")
                        nc.tensor.matmul(m_ps[gl][:, 0:129], C["ident_b"][:],
                                         C["ident130_b"][:, 0:129],
                                         start=True, stop=False)
                        for j in sorted(range(16), key=lambda j: (j in pool_js, j)):
                            nc.tensor.matmul(m_ps[gl][:, 0:129], soh[gl][:, j, :],
                                             ta[gl][:, j, 0:129],
                                             start=False, stop=(j == 15))
                    xe8 = wp.tile([128, gpb], F32, tag="xe8", name="xe8", bufs=3)
                    o4c8 = wp.tile([128, gpb], F32, tag="o4c8", name="o4c8", bufs=3)
                    deg8_ps = psQ.tile([128, 32], F32, tag="Qp", name="deg8ps")
                    for gl in G:
                        mbar[gl] = wp.tile([128, 128], F32, tag="mbar", name=f"mbar{gl}", bufs=gpb + 2)
                        nc.scalar.activation(mbar[gl][:], m_ps[gl][:, 0:128], AF.Copy)
                        nc.scalar.activation(xe8[:, gl:gl + 1], m_ps[gl][:, 128:129], AF.Copy)
                        nc.tensor.matmul(deg8_ps[:, gl:gl + 1], mbar[gl][:], C["ones_col"][:],
                                         start=True, stop=True)
                    rec8 = wp.tile([128, gpb], F32, tag="rec8", name="rec8", bufs=3)
                    nc.vector.reciprocal(rec8[:], deg8_ps[:, 0:gpb])
                    dis8 = wp.tile([128, gpb], F32, tag="dis8", name="dis8", bufs=3)
                    nc.scalar.activation(dis8[:], rec8[:], AF.Sqrt)
                    for gl in G:
                        dis[gl] = dis8[:, gl:gl + 1]
                        nc.gpsimd.tensor_scalar(mbar[gl][:], mbar[gl][:], dis[gl], None,
                                                op0=OP.mult)
                    BW = 4 if gpb % 4 == 0 else (2 if gpb % 2 == 0 else 1)
                    NB = gpb // BW
                    for gl in G:
                        xt_ps[gl] = psT.tile([128, 128], F32, tag="T", name=f"xtps{gl}")
                        nc.tensor.transpose(xt_ps[gl][:], xg[gl], C["ident"][:])
                        xt[gl] = wp.tile([128, 128], F32, tag="xt", name=f"xt{gl}", bufs=gpb + 2)
                        nc.scalar.activation(xt[gl][:], xt_ps[gl][:], AF.Copy)
                    # batched transpose of all xe columns -> one [gpb,128] copy
                    xet8_ps = psT.tile([128, 128], F32, tag="T", name="xet8ps")
                    nc.tensor.transpose(xet8_ps[0:gpb, :], xe8[:], C["ident"][:])
                    xet8 = wp.tile([gpb, 128], F32, tag="xet8", name="xet8", bufs=3)
                    nc.scalar.activation(xet8[:], xet8_ps[0:gpb, :], AF.Copy)
                    # stage: P1 (raw, no dis scaling -- dis lives in mbar rows)
                    phat4 = {}
                    for b in range(NB):
                        p_ps = psP.tile([128, 32 * BW], F32, tag="Pp", name=f"pps{b}")
                        for m in range(BW):
                            gl = b * BW + m
                            nc.tensor.matmul(p_ps[:, 32 * m:32 * m + 32], xt[gl][:],
                                             C["W1a"][:], start=True, stop=False)
                            nc.tensor.matmul(p_ps[:, 32 * m:32 * m + 32], xet8[:],
                                             C["W1b_sel"][:, gl, :], start=False, stop=True)
                        phat4[b] = wp.tile([128, 32 * BW], F32, tag="phat", name=f"phat{b}", bufs=gpb + 2)
                        nc.scalar.activation(phat4[b][:], p_ps[:], AF.Copy)
                    # layers, stage-interleaved across graphs
                    pw = [C["W2"][:], C["Wpad"][32:64, 0:32], C["Wpad"][64:96, 0:1]]
                    bcols = [C["b1c"], C["b2c"], C["b3c"]]
                    oitall = {}
                    for b in range(NB):
                        oitall[b] = wp.tile([128, 128 * BW], F32, tag="oitall",
                                            name=f"oitall{b}", bufs=NB + 1)
                        nc.gpsimd.memset(oitall[b][96:97, :], 1.0)
                    q_ps, q1 = {}, {}
                    for li in range(4):
                        w = 32 if li < 3 else 1
                        for gl in G:
                            b, m = gl // BW, gl % BW
                            q_ps[gl] = psQ.tile([128, 32], F32, tag="Qp", name=f"qps{li}_{gl}")
                            nc.tensor.matmul(q_ps[gl][:, 0:w], mbar[gl][:],
                                             phat4[b][:, w * m:w * m + w],
                                             start=True, stop=True)
                            if li < 3:
                                q1[gl] = wp.tile([128, 32], F32, tag="q1", name=f"q1_{li}_{gl}", bufs=gpb + 2)
                                nc.scalar.activation(q1[gl][:, 0:w], q_ps[gl][:, 0:w], AF.Copy,
                                                     scale=dis[gl])
                            else:
                                # o4 (sans +b4: constant shift, rank-invariant)
                                # into the group score-column tile
                                nc.scalar.activation(o4c8[:, gl:gl + 1],
                                                     q_ps[gl][:, 0:1], AF.Copy,
                                                     scale=dis[gl])
                        if li < 3:
                            for b in range(NB):
                                q1t_ps = psT.tile([32, 128 * BW], F32, tag="T", name=f"q1tps{li}_{b}")
                                for m in range(BW):
                                    nc.tensor.transpose(q1t_ps[0:32, 128 * m:128 * m + 128],
                                                        q1[b * BW + m][:, 0:32], C["ident"][:])
                                nc.scalar.activation(oitall[b][32 * li:32 * li + 32, :],
                                                     q1t_ps[:], AF.Identity,
                                                     bias=bcols[li][:])
                            for b in range(NB):
                                p_ps2 = psP.tile([128, 32 * BW], F32, tag="Pp", name=f"pps2{li}_{b}")
                                wnext = 32 if li < 2 else 1
                                for m in range(BW):
                                    nc.tensor.matmul(p_ps2[:, wnext * m:wnext * m + wnext],
                                                     oitall[b][32 * li:32 * li + 32, 128 * m:128 * m + 128],
                                                     pw[li], start=True, stop=True)
                                phat4[b] = wp.tile([128, 32 * BW], F32, tag="phat", name=f"phatL{li}_{b}", bufs=gpb + 2)
                                nc.scalar.activation(phat4[b][:, 0:wnext * BW], p_ps2[:, 0:wnext * BW], AF.Copy)
                        else:
                            # batched transpose of the gpb o4 columns, one
                            # bias-adding copy for all graphs in the group
                            o4t8_ps = psT.tile([128, 128], F32, tag="T", name="o4t8ps")
                            nc.tensor.transpose(o4t8_ps[0:gpb, :], o4c8[:],
                                                C["ident"][:])
                            o4t8 = wp.tile([gpb, 128], F32, tag="o4t8", name="o4t8", bufs=3)
                            nc.scalar.activation(o4t8[:], o4t8_ps[0:gpb, :],
                                                 AF.Identity, bias=C["b4rep"][0:gpb, :])
                    # stage: c1 over all nodes + relu (one stacked matmul:
                    # 96 layer-feature rows + ones/bias row, plus o4 select)
                    for gl in G:
                        b, m = gl // BW, gl % BW
                        c1_ps = psQ.tile([128, 32], F32, tag="Qp", name=f"c1ps{gl}")
                        nc.tensor.matmul(c1_ps[:, 0:16],
                                         oitall[b][0:97, 128 * m:128 * m + 128],
                                         C["c1cat"][:], start=True, stop=False)
                        nc.tensor.matmul(c1_ps[:, 0:16], o4t8[:],
                                         C["c1w96_sel"][:, gl, :], start=False, stop=True)
                        nc.scalar.activation(c1all[:, bass.ds(i * gpb + gl, 1), :],
                                             c1_ps[:, 0:16], AF.Relu)

                    # ---- fused per-group top-k + select + maxpool + conv ----
                    scur = o4t8
                    idxf8 = wp.tile([gpb, 24], F32, tag="idxf8", name="idxf8", bufs=3)
                    for r in range(3):
                        mx = wp.tile([gpb, 8], F32, tag=f"mx{r}", name=f"mx{r}", bufs=3)
                        mi = wp.tile([gpb, 8], U32, tag=f"mi{r}", name=f"mi{r}", bufs=3)
                        nc.vector.max(mx[:], scur[:])
                        nc.vector.max_index(mi[:], mx[:], scur[:])
                        nc.vector.tensor_copy(idxf8[:, 8 * r:8 * r + 8], mi[:])
                        if r < 2:
                            snew = wp.tile([gpb, 128], F32, tag=f"sc{r}", name=f"sc{r}", bufs=3)
                            nc.vector.match_replace(snew[:], mx[:], scur[:], -1e30)
                            scur = snew
                    idxt8_ps = psT.tile([128, 128], F32, tag="T", name="idxt8ps")
                    nc.tensor.transpose(idxt8_ps[0:24, 0:gpb], idxf8[:],
                                        C["ident"][0:gpb, 0:gpb])
                    idxt8 = wp.tile([K, gpb], F32, tag="idxt8", name="idxt8", bufs=3)
                    nc.scalar.activation(idxt8[:], idxt8_ps[0:K, 0:gpb], AF.Copy)
                    selT, sel = {}, {}
                    for gl in G:
                        selT[gl] = wp.tile([K, 128], F32, tag="selT", name=f"selT{gl}", bufs=gpb + 2)
                        nc.vector.tensor_scalar(selT[gl][:], C["iota_f"][0:K, :],
                                                idxt8[:, gl:gl + 1], None,
                                                op0=OP.is_equal)
                    for gl in G:
                        sel_ps = psT.tile([128, 128], F32, tag="T", name=f"selps{gl}")
                        nc.tensor.transpose(sel_ps[:, 0:K], selT[gl][:], C["ident"][0:K, 0:K])
                        sel[gl] = wp.tile([128, K], F32, tag="sel", name=f"sel{gl}", bufs=gpb + 2)
                        nc.scalar.activation(sel[gl][:], sel_ps[:, 0:K], AF.Copy)
                    # pooled features, already transposed: [16, K] per graph
                    c1st_all = wp.tile([16, gpb, K], F32, tag="c1st", name="c1st_all", bufs=3)
                    for gl in G:
                        c1st_ps = psQ.tile([128, 32], F32, tag="Qp", name=f"c1stps{gl}")
                        nc.tensor.matmul(c1st_ps[0:16, 0:K],
                                         c1all[:, bass.ds(i * gpb + gl, 1), :],
                                         sel[gl][:], start=True, stop=True)
                        nc.scalar.activation(c1st_all[:, gl, :], c1st_ps[0:16, 0:K], AF.Copy)
                    # maxpool over pairs for all graphs in the group at once
                    mp_all = wp.tile([16, gpb, K // 2], F32, tag="mp", name="mp_all", bufs=3)
                    nc.vector.tensor_tensor(mp_all[:], c1st_all[:, :, 0:K:2],
                                            c1st_all[:, :, 1:K:2], op=OP.max)
                    # conv1d over the group: 5 batched matmuls
                    c2_ps = psP.tile([32, gpb, 6], F32, tag="Pp", name="c2ps")
                    for k5 in range(5):
                        nc.tensor.matmul(c2_ps[:], C["c2wT"][:, k5, :],
                                         mp_all[:, :, k5:k5 + 6],
                                         start=(k5 == 0), stop=(k5 == 4))
                    nc.scalar.activation(C2all[:, bass.ds(i * gpb, gpb), :], c2_ps[:],
                                         AF.Relu, bias=C["c2b_col"][:])

                if unroll_static:
                    for i in range(groups):
                        body1(i)
                else:
                    with tc.For_i(0, groups, 1) as i:
                        body1(i)

            # ---------------- batched MLP head ----------------
            with tc.tile_pool(name="mlp", bufs=1) as ml, \
                 tc.tile_pool(name="psD", bufs=2, space="PSUM") as psD, \
                 tc.tile_pool(name="psE", bufs=2, space="PSUM") as psE:
                d1_ps = [psD.tile([128, 512], F32, tag="d1", name=f"d1ps{h}") for h in range(2)]
                for h in range(2):
                    for t in range(6):
                        nc.tensor.matmul(d1_ps[h][0:gpc, :], C2all[:, :, t],
                                         C["outw_r"][:, t, 512 * h:512 * h + 512],
                                         start=(t == 0), stop=False)
                    nc.tensor.matmul(d1_ps[h][0:gpc, :], C["ones_row"][:, 0:gpc],
                                     C["outb_row"][:, 512 * h:512 * h + 512],
                                     start=False, stop=True)
                d1r = ml.tile([128, 1024], F32, tag="d1r")
                for h in range(2):
                    nc.scalar.activation(d1r[0:gpc, 512 * h:512 * h + 512],
                                         d1_ps[h][0:gpc, :], AF.Relu)
                d2_ps = psD.tile([128, 512], F32, tag="d1")
                for u in range(8):
                    t_ps = psE.tile([128, 128], F32, tag="tr")
                    nc.tensor.transpose(t_ps[0:128, 0:gpc],
                                        d1r[0:gpc, 128 * u:128 * u + 128],
                                        C["ident"][0:gpc, 0:gpc])
                    d1tu = ml.tile([128, 128], F32, tag=f"d1t{u % 2}")
                    nc.scalar.activation(d1tu[:, 0:gpc], t_ps[:, 0:gpc], AF.Copy)
                    nc.tensor.matmul(d2_ps[:, 0:gpc], C["h1w"][:, u, :],
                                     d1tu[:, 0:gpc], start=(u == 0), stop=False)
                nc.tensor.matmul(d2_ps[:, 0:gpc], C["h1b_row"][:], C["ones_row"][:, 0:gpc],
                                 start=False, stop=True)
                d2t = ml.tile([128, 128], F32, tag="d2t")
                nc.scalar.activation(d2t[:, 0:gpc], d2_ps[:, 0:gpc], AF.Relu)
                lg_ps = psE.tile([128, 128], F32, tag="tr")
                nc.tensor.matmul(lg_ps[0:gpc, 0:16], d2t[:, 0:gpc], C["h2w"][:],
                                 start=True, stop=False)
                nc.tensor.matmul(lg_ps[0:gpc, 0:16], C["ones_row"][:, 0:gpc],
                                 C["h2b_row"][:], start=False, stop=True)
                # log_softmax over the 16 logits
                negmax = ml.tile([128, 1], F32, tag="negmax")
                nc.vector.tensor_reduce(negmax[0:gpc, :], lg_ps[0:gpc, 0:16], axis=AX.X,
                                        op=OP.max, negate=True)
                ex = ml.tile([128, 16], F32, tag="ex")
                nc.scalar.activation(ex[0:gpc, :], lg_ps[0:gpc, 0:16], AF.Exp,
                                     bias=negmax[0:gpc, :])
                ssum = ml.tile([128, 1], F32, tag="ssum")
                nc.vector.tensor_reduce(ssum[0:gpc, :], ex[0:gpc, :], axis=AX.X, op=OP.add)
                lsum = ml.tile([128, 1], F32, tag="lsum")
                nc.scalar.activation(lsum[0:gpc, :], ssum[0:gpc, :], AF.Ln)
                mpls = ml.tile([128, 1], F32, tag="mpls")
                nc.vector.tensor_sub(mpls[0:gpc, :], lsum[0:gpc, :], negmax[0:gpc, :])
                outsb = ml.tile([128, 16], F32, tag="outsb")
                nc.vector.tensor_scalar(outsb[0:gpc, :], lg_ps[0:gpc, 0:16],
                                        mpls[0:gpc, :], None, op0=OP.subtract)
                nc.sync.dma_start(out_d[:], outsb[0:gpc, :])

    if do_compile:
        nc.compile()
    return nc


def shard_inputs(inputs, gpc=128, gpb=8):
    """Slice full inputs into 8 per-core input maps (layout packing only)."""
    x = np.asarray(inputs["x"], np.float32)
    ea = np.asarray(inputs["edge_attr"], np.float32).reshape(-1)
    ei = np.asarray(inputs["edge_index"], np.int32)
    consts = _constants(inputs)
    groups = gpc // gpb
    epc = gpc * EPG
    maps = []
    for c in range(NCORES):
        m = dict(consts)
        xc = x[c * gpc * P:(c + 1) * gpc * P]            # [gpc*128, 128]
        m["xs"] = np.ascontiguousarray(
            xc.reshape(gpc, P, NF).transpose(1, 0, 2).reshape(P, gpc * NF))
        for name, arr in (("rowt", ei[0]), ("colt", ei[1]), ("attrt", ea)):
            sl = arr[c * epc:(c + 1) * epc]
            # [groups, gpb, 16, 128] -> [groups, 128, gpb*16]
            t = sl.reshape(groups, gpb * 16, 128).transpose(0, 2, 1)
            m[name] = np.ascontiguousarray(t.reshape(groups * 128, gpb * 16))
        maps.append(m)
    return maps


_CACHE = {}


def kernel(**inputs) -> np.ndarray:
    gpc = 128
    key = ("full", gpc)
    if key not in _CACHE:
        _CACHE[key] = build(gpc=gpc)
    nc = _CACHE[key]
    maps = shard_inputs(inputs, gpc=gpc)
    res = run_bass_kernel_spmd(nc, maps, core_ids=list(range(NCORES)))
    return np.concatenate([res.results[i]["out"] for i in range(NCORES)], axis=0)

